# revision 1
# baseline (speedup 1.0000x reference)
"""Trainium2 Bass kernel for nn_ClusteringLayer (VQ codebook assign + gather).

reference:
    flat = x.reshape(B, H*W, C)
    dists[b,n,k] = ||flat[b,n]||^2 - 2 flat[b,n].c_k + ||c_k||^2
    idx = argmin_k dists ; y = centers[idx]
    returns (x, y)

Strategy (8 NeuronCores, data-parallel over batch):
  - Each core gets 4 of the 32 batches = 16384 tokens of 256 dims.
  - scores[t,k] = x_t.c_k - 0.5||c_k||^2 computed on the PE:
    tokens on PSUM partitions, centers on the free dim. Contraction (C=256)
    split into 2 chunks of 128. Matmuls in float32r (full-rate PE path,
    ~TF32 precision) plus an exact bf16 hi/lo bias matmul (K=2) that adds
    -0.5||c_k||^2.
  - ACT copies PSUM->SBUF; DVE max8 + max_index give the top-8 scores and
    the argmax index per token.
  - GPSIMD indirect DMA gathers centers[idx] (the VQ table lookup) on
    device; y is written back with one DMA per 512-token slab.
  - float32r error (measured <= 1.3e-2) can flip the argmax only when the
    top-2 margin is tiny. The device exports the top-8 scores; the host
    re-scores the ~1% of tokens whose margin < 0.0625 exactly in fp32 and
    patches those y rows. Result matches a full-fp32 argmin exactly.
"""
from contextlib import ExitStack

import numpy as np
import ml_dtypes

import concourse.bass as bass
import concourse.bacc as bacc
import concourse.mybir as mybir
import concourse.tile as tile
import concourse.bass_utils as bass_utils

# problem shape (hardcoded per contest contract)
B, H, W, C = 32, 64, 64, 256
K = 512
N_CORES = 8
P = 128
NTOK = B * H * W // N_CORES  # 16384 tokens per core

R = mybir.dt.float32r
BF = mybir.dt.bfloat16
F32 = mybir.dt.float32
U32 = mybir.dt.uint32

TILES_PER_SLAB = 4
PSUM_BUFS = 8
FIXUP_DELTA = 0.0625

_NC_CACHE = {}


def _build(ntok: int, num_devices: int):
    ntiles = ntok // P
    nslab = ntiles // TILES_PER_SLAB

    nc = bacc.Bacc("TRN2", target_bir_lowering=False, debug=False,
                   num_devices=num_devices)
    xT_d = nc.dram_tensor("xT", [C, ntok], R, kind="ExternalInput").ap()
    cT_d = nc.dram_tensor("cT", [C, K], R, kind="ExternalInput").ap()
    b2_d = nc.dram_tensor("bias2", [2, K], BF, kind="ExternalInput").ap()
    o2_d = nc.dram_tensor("ones2", [2, P], BF, kind="ExternalInput").ap()
    cent_d = nc.dram_tensor("centers", [K, C], F32, kind="ExternalInput").ap()
    y_d = nc.dram_tensor("y", [ntok, C], F32, kind="ExternalOutput").ap()
    val8_d = nc.dram_tensor("val8", [ntok, 8], F32, kind="ExternalOutput").ap()

    xT_v = xT_d.rearrange("(h p) n -> p h n", h=2)
    y_slab = y_d.rearrange("(t p) c -> p t c", p=P)

    with tile.TileContext(nc) as tc, ExitStack() as ctx:
        constp = ctx.enter_context(tc.tile_pool(name="const", bufs=1))
        xp = ctx.enter_context(tc.tile_pool(name="x", bufs=4))
        scp = ctx.enter_context(tc.tile_pool(name="sc", bufs=8))
        yp = ctx.enter_context(tc.tile_pool(name="y", bufs=4))
        mxp = ctx.enter_context(tc.tile_pool(name="mx", bufs=8))
        accp = ctx.enter_context(tc.tile_pool(name="acc", bufs=1))
        psump = ctx.enter_context(
            tc.tile_pool(name="psum", bufs=PSUM_BUFS, space="PSUM"))

        ct0 = constp.tile([P, K], R, tag="ct0")
        ct1 = constp.tile([P, K], R, tag="ct1")
        nc.sync.dma_start(ct0[:], cT_d[0:P, :])
        nc.sync.dma_start(ct1[:], cT_d[P:2 * P, :])
        b2 = constp.tile([2, K], BF, tag="b2")
        nc.sync.dma_start(b2[:], b2_d[:])
        o2 = constp.tile([2, P], BF, tag="o2")
        nc.sync.dma_start(o2[:], o2_d[:])

        val8_a = accp.tile([P, ntiles, 8], F32, tag="val8a")
        idx8_a = accp.tile([P, ntiles, 8], U32, tag="idx8a")
        val8_b = accp.tile([P, ntiles, 8], F32, tag="val8b")
        idx8_b = accp.tile([P, ntiles, 8], U32, tag="idx8b")

        SL = TILES_PER_SLAB * P
        for s in range(nslab):
            xs = xp.tile([P, 2, SL], R, tag="xs")
            nc.sync.dma_start(xs[:], xT_v[:, :, bass.ts(s, SL)])

            yg = yp.tile([P, TILES_PER_SLAB, C], F32, tag="yg")
            for j in range(TILES_PER_SLAB):
                t = s * TILES_PER_SLAB + j
                sc = scp.tile([P, K], F32, tag="sc")
                ps = psump.tile([P, K], F32, tag="ps")
                nc.tensor.matmul(ps[:], o2[:], b2[:], start=True, stop=False)
                nc.tensor.matmul(ps[:], xs[:, 0, bass.ts(j, P)], ct0[:],
                                 start=False, stop=False)
                nc.tensor.matmul(ps[:], xs[:, 1, bass.ts(j, P)], ct1[:],
                                 start=False, stop=True)
                nc.scalar.copy(sc[:], ps[:])

                va = val8_b if t % 2 else val8_a
                ia = idx8_b if t % 2 else idx8_a
                nc.vector.max(va[:, t, :], sc[:])
                nc.vector.max_index(ia[:, t, :], va[:, t, :], sc[:])

            for j in range(TILES_PER_SLAB):
                t = s * TILES_PER_SLAB + j
                nc.gpsimd.indirect_dma_start(
                    out=yg[:, j, :],
                    out_offset=None,
                    in_=cent_d[:],
                    in_offset=bass.IndirectOffsetOnAxis(
                        ap=(idx8_b if t % 2 else idx8_a)[:, t, 0:1], axis=0),
                )
            nc.sync.dma_start(y_slab[:, bass.ts(s, TILES_PER_SLAB), :], yg[:])

        v8v = val8_d.rearrange("(u v p) e -> v p u e", v=2, p=P)
        nc.sync.dma_start(
            v8v[0], val8_a[:].rearrange("p (u v) e -> v p u e", v=2)[0])
        nc.sync.dma_start(
            v8v[1], val8_b[:].rearrange("p (u v) e -> v p u e", v=2)[1])

    nc.compile()
    return nc


def _host_prep(x_core: np.ndarray, centers: np.ndarray, shared: dict):
    xT = np.ascontiguousarray(x_core.T)
    return {"xT": xT, **shared}


def _shared_inputs(centers: np.ndarray):
    bf16 = ml_dtypes.bfloat16
    cT = np.ascontiguousarray(centers.T)
    c_sq = (centers.astype(np.float64) ** 2).sum(-1)
    bias = (-0.5 * c_sq).astype(np.float32)
    bias_hi = bias.astype(bf16)
    bias_lo = (bias - bias_hi.astype(np.float32)).astype(bf16)
    bias2 = np.concatenate([bias_hi[None, :], bias_lo[None, :]], axis=0)
    ones2 = np.ones((2, P), dtype=bf16)
    return {"cT": cT, "bias2": bias2, "ones2": ones2, "centers": centers}


def _host_fixup(x_core, centers, c_sq32, y, val8, delta=FIXUP_DELTA):
    """Rescore tokens whose device top-2 margin < delta exactly in fp32."""
    gap = val8[:, 0] - val8[:, 1]
    flag = gap < delta
    if flag.any():
        xf = x_core[flag]
        d = c_sq32[None, :] - 2.0 * (xf @ centers.T)
        y[flag] = centers[d.argmin(-1)]
    return y


def kernel(x: np.ndarray, centers: np.ndarray):
    x = np.asarray(x)
    centers = np.ascontiguousarray(np.asarray(centers, dtype=np.float32))
    assert x.shape == (B, H, W, C) and centers.shape == (K, C)

    key = (NTOK, N_CORES)
    if key not in _NC_CACHE:
        _NC_CACHE[key] = _build(NTOK, N_CORES)
    nc = _NC_CACHE[key]

    shared = _shared_inputs(centers)
    flat = np.ascontiguousarray(x, dtype=np.float32).reshape(N_CORES, NTOK, C)
    in_maps = [_host_prep(flat[c], centers, shared) for c in range(N_CORES)]

    res = bass_utils.run_bass_kernel_spmd(nc, in_maps, core_ids=list(range(N_CORES)))

    c_sq32 = (centers * centers).sum(-1)
    y = np.empty((N_CORES, NTOK, C), dtype=np.float32)
    for c in range(N_CORES):
        yc = res.results[c]["y"].copy()
        val8 = res.results[c]["val8"]
        y[c] = _host_fixup(flat[c], centers, c_sq32, yc, val8)

    return (x, y.reshape(B, H, W, C))



# revision 2
# speedup vs baseline: 1.3876x; 1.3876x over previous
"""Trainium2 Bass kernel v3: u8 score export + tri-engine PSUM drain.

Device: per core 128 token-tiles; 2 bf16 matmuls/tile -> PSUM fp32 raw scores
x.c_k; drains rotate over ACT/DVE/GPSIMD converting to u8 (q = s*SCALE + 128,
monotone per-tile); DMA exports 8.4MB of u8 scores (+8.4MB bf16 x in) so the
DMA_ENGINES aggregate stays under the PE floor.

Host: dequantize, add -0.5||c||^2 bias, argmax, flag small-margin/saturated
tokens, rescore them exactly in fp32, gather y = centers[idx].
"""
from contextlib import ExitStack

import numpy as np
import ml_dtypes

import concourse.bass as bass
import concourse.bacc as bacc
import concourse.mybir as mybir
import concourse.tile as tile
import concourse.bass_utils as bass_utils

B, H, W, C = 32, 64, 64, 256
K = 512
N_CORES = 8
P = 128
NTOK = B * H * W // N_CORES  # 16384

BF = mybir.dt.bfloat16
F32 = mybir.dt.float32
U8 = mybir.dt.uint8

GROUP = 4

SCALE = 1.22    # u8 = round(s * SCALE) + 128; |s| <= ~104 assumed (6.5 sigma)
OFFSET = 128.0
# flag threshold in dequantized units: covers bf16-matmul err + u8 rounding
FIXUP_DELTA = 2.2

_NC_CACHE = {}


def _build(ntok: int, num_devices: int):
    ntiles = ntok // P
    ngroup = ntiles // GROUP

    nc = bacc.Bacc("TRN2", target_bir_lowering=False, debug=False,
                   num_devices=num_devices)
    xT_d = nc.dram_tensor("xT", [C, ntok], BF, kind="ExternalInput").ap()
    cT_d = nc.dram_tensor("cT", [C, K], BF, kind="ExternalInput").ap()
    sc_d = nc.dram_tensor("scores", [ntok, K], U8, kind="ExternalOutput").ap()

    xT_v = xT_d.rearrange("(h p) n -> p h n", h=2)
    sc_v = sc_d.rearrange("(a p) k -> p a k", p=P)

    SL = GROUP * P

    with tile.TileContext(nc) as tc, ExitStack() as ctx:
        constp = ctx.enter_context(tc.tile_pool(name="const", bufs=1))
        xp = ctx.enter_context(tc.tile_pool(name="x", bufs=6))
        scp = ctx.enter_context(tc.tile_pool(name="sc", bufs=6))
        psump = ctx.enter_context(
            tc.tile_pool(name="psum", bufs=2, space="PSUM"))

        ct0 = constp.tile([P, K], BF, tag="ct0")
        ct1 = constp.tile([P, K], BF, tag="ct1")
        nc.sync.dma_start(ct0[:], cT_d[0:P, :])
        nc.sync.dma_start(ct1[:], cT_d[P:2 * P, :])
        off = constp.tile([P, 1], F32, tag="off")
        nc.vector.memset(off[:], OFFSET)

        for g in range(ngroup):
            xs = xp.tile([P, 2, SL], BF, tag="xs")
            nc.sync.dma_start(xs[:], xT_v[:, :, bass.ts(g, SL)])

            ps = psump.tile([P, GROUP, K], F32, tag="ps")
            for j in range(GROUP):
                nc.tensor.matmul(ps[:, j, :], xs[:, 0, bass.ts(j, P)], ct0[:],
                                 start=True, stop=False)
                nc.tensor.matmul(ps[:, j, :], xs[:, 1, bass.ts(j, P)], ct1[:],
                                 start=False, stop=True)

            sc8 = scp.tile([P, GROUP, K], U8, tag="sc8")
            r = g % 2
            if r == 0:
                nc.scalar.activation(sc8[:], ps[:],
                                     mybir.ActivationFunctionType.Identity,
                                     bias=off[:], scale=SCALE)
            elif r == 1:
                nc.vector.tensor_scalar(sc8[:], ps[:], SCALE, OFFSET,
                                        op0=mybir.AluOpType.mult,
                                        op1=mybir.AluOpType.add)
            else:
                nc.gpsimd.tensor_scalar(sc8[:], ps[:], SCALE, OFFSET,
                                        op0=mybir.AluOpType.mult,
                                        op1=mybir.AluOpType.add)

            nc.sync.dma_start(sc_v[:, bass.ts(g, GROUP), :], sc8[:])

    nc.compile()
    return nc


def _host_postprocess(flat32, centers, scores_u8, c_sq, delta=FIXUP_DELTA):
    sc = scores_u8.astype(np.float32)
    sc -= OFFSET
    sc *= (1.0 / SCALE)
    sc -= 0.5 * c_sq[None, :]
    idx = np.argmax(sc, axis=-1)
    n = sc.shape[0]
    ar = np.arange(n)
    m1 = sc[ar, idx]
    sat = scores_u8[ar, idx] >= 254
    sc[ar, idx] = -np.inf
    m2 = sc.max(axis=-1)
    flag = ((m1 - m2) < delta) | sat
    if flag.any():
        xf = flat32[flag]
        d = c_sq[None, :] - 2.0 * (xf @ centers.T)
        idx[flag] = d.argmin(-1)
    return idx


def kernel(x: np.ndarray, centers: np.ndarray):
    x = np.asarray(x)
    centers = np.ascontiguousarray(np.asarray(centers, dtype=np.float32))
    assert x.shape == (B, H, W, C) and centers.shape == (K, C)

    key = (NTOK, N_CORES)
    if key not in _NC_CACHE:
        _NC_CACHE[key] = _build(NTOK, N_CORES)
    nc = _NC_CACHE[key]

    bf16 = ml_dtypes.bfloat16
    cT = np.ascontiguousarray(centers.T).astype(bf16)
    flat32 = np.ascontiguousarray(x, dtype=np.float32).reshape(N_CORES, NTOK, C)
    in_maps = []
    for c in range(N_CORES):
        xT = np.ascontiguousarray(flat32[c].T).astype(bf16)
        in_maps.append({"xT": xT, "cT": cT})

    res = bass_utils.run_bass_kernel_spmd(nc, in_maps,
                                          core_ids=list(range(N_CORES)))

    c_sq = (centers * centers).sum(-1)
    idx = np.empty((N_CORES, NTOK), dtype=np.int64)
    for c in range(N_CORES):
        scores = res.results[c]["scores"]
        idx[c] = _host_postprocess(flat32[c], centers, scores, c_sq)

    y = centers[idx.reshape(-1)].reshape(B, H, W, C)
    return (x, y)
